# revision 2
# baseline (speedup 1.0000x reference)
"""Trainium2 Bass kernel for nn_BasicLSTM: (B,T,N,C) shared-weight LSTM -> FC.

Strategy (data parallel over 8 cores, B=64 -> 8 batches/core):
  - seqs = 8*1370 = 10960 (padded to 11264) independent (b,n) sequences per
    core, T=12, C=8, H=64.
  - "Gates on partitions, sequences on free dim". Per step t, per PAIR of
    512-seq blocks, 8 matmuls (4 gates x 2 blocks) with stationary
    lhsT = [W_hh.T; W_ih.T; b] (73 x 64) compute gate pre-activations into
    gate-major PSUM (block0 -> partitions 0:64, block1 -> 64:128).
  - Pairs are processed in COUPLES (2 pairs = 2048 seqs). All SBUF-side
    tensors (sigmoid/tanh outputs, cell state, h) live in couple-wide
    supertiles so the DVE/Pool/ACT ops run at 1024-2048 columns per
    instruction, halving per-instruction overhead.
  - ACT: sigmoid over each pair's [i|f|o] PSUM banks, tanh per pair's g bank,
    one couple-wide tanh on the cell state.
  - DVE: i*g, c=ig+fc, h-writes (even blocks written straight into the next
    step's rhs supertile via strided APs; odd blocks staged + 1 DMA).
  - GpSimd (Pool): f*c multiply, couple-wide, plus the per-couple-step x DMA.
  - x arrives pre-transposed from host as (9, T, seqs_pad) bf16 (channel 8 is
    ones, carrying the fused bias through the contraction).
  - FC tail: y = W_fc @ h + b_fc as K=65 matmuls over the stored final h.
"""

import os
from contextlib import ExitStack

import numpy as np

import concourse.bass as bass
import concourse.mybir as mybir
import concourse.tile as tile
from concourse import bacc
from concourse.bass_utils import run_bass_kernel_spmd

B, T, N, C, H = 64, 12, 1370, 8, 64
NCORES = 8
BPC = B // NCORES          # batches per core
SEQS = BPC * N             # 10960 sequences per core
S = 512                    # block size (free dim per matmul)
NBLK = (SEQS + S - 1) // S # 22
SEQS_PAD = NBLK * S        # 11264
KH = H                     # 64 rows of h in rhs
KX = C + 1                 # 8 x-channels + ones row
K = KH + KX                # 73
G4 = 4 * H                 # 256

BF16 = mybir.dt.bfloat16
F32 = mybir.dt.float32
CDT = BF16
NPBF16 = mybir.dt.np(BF16)

AF = mybir.ActivationFunctionType

# couples: list of (block0, npairs); last couple may have 1 pair
_COUPLES = []
_b = 0
while _b < NBLK:
    np_ = 2 if _b + 4 <= NBLK else 1
    _COUPLES.append((_b, np_))
    _b += 2 * np_

GROUP = 3      # couples per interleave group
PREFETCH = 3   # rhs supertiles prefetched ahead


def build_nc() -> bass.Bass:
    nc = bacc.Bacc("TRN2", target_bir_lowering=False, debug=False)

    xin = nc.declare_dram_parameter("xin", [KX, T, SEQS_PAD], BF16, isOutput=False)
    wg = nc.declare_dram_parameter("wg", [K, G4], BF16, isOutput=False)
    wfc = nc.declare_dram_parameter("wfc", [H + 1, C], BF16, isOutput=False)
    y = nc.declare_dram_parameter("y", [C, SEQS_PAD], F32, isOutput=True)

    with tile.TileContext(nc) as tc, ExitStack() as ctx:
        const = ctx.enter_context(tc.tile_pool(name="const", bufs=1))
        rhsp = ctx.enter_context(tc.tile_pool(name="rhs", bufs=20))
        sgp = ctx.enter_context(tc.tile_pool(name="sg", bufs=5))
        thgp = ctx.enter_context(tc.tile_pool(name="thg", bufs=5))
        thcp = ctx.enter_context(tc.tile_pool(name="thc", bufs=5))
        igp = ctx.enter_context(tc.tile_pool(name="ig", bufs=4))
        fcpool = ctx.enter_context(tc.tile_pool(name="fcp", bufs=4))
        cpool = ctx.enter_context(tc.tile_pool(name="cst", bufs=5))
        htp = ctx.enter_context(tc.tile_pool(name="htmp", bufs=4))
        ysp = ctx.enter_context(tc.tile_pool(name="ys", bufs=4))
        pifo = ctx.enter_context(tc.tile_pool(name="pifo", bufs=2, space="PSUM"))
        pgp = ctx.enter_context(tc.tile_pool(name="pg", bufs=2, space="PSUM"))

        w_sb = const.tile([K, G4], BF16)
        nc.sync.dma_start(out=w_sb[:, :], in_=wg[:, :])
        wfc_sb = const.tile([H + 1, C], BF16)
        nc.sync.dma_start(out=wfc_sb[:, :], in_=wfc[:, :])
        hstore = const.tile([H + 1, SEQS_PAD], BF16)
        # ones row from xin's ones channel
        nc.sync.dma_start(out=hstore[H : H + 1, :], in_=xin[C : C + 1, 0, :])
        # PE warm-up + early ACT table load under the initial DMA shadow
        scratch = const.tile([128, S], BF16)
        nc.vector.memset(scratch[:, :], 1.0)
        nc.scalar.activation(scratch[0:1, 0:8], scratch[0:1, 0:8], AF.Sigmoid)

        WI, WF, WG, WO = 0, H, 2 * H, 3 * H

        def emit_step(st, t):
            cb0, npair = st["cb0"], st["np"]
            W = npair * 2 * S            # couple rhs width
            Wg = npair * S               # couple per-gate width
            rhs_t, c_t = st["rhs"], st["c"]

            sg = sgp.tile([128, npair * 3 * S], BF16, name="sg", tag="sg")
            thg = thgp.tile([128, Wg], BF16, name="thg", tag="thg")

            for p in range(npair):
                ifo = pifo.tile([128, 3 * S], F32, name="ifo", tag="ifo")
                pg = pgp.tile([128, S], F32, name="pg", tag="pg")
                # matmuls; g gate first so its tanh can start early
                for gof, dst, dsec in (
                    (WG, pg, 0),
                    (WI, ifo, 0),
                    (WF, ifo, 1),
                    (WO, ifo, 2),
                ):
                    for blk in range(2):
                        col = (2 * p + blk) * S
                        pb = 64 * blk
                        if t == 0:
                            lh = w_sb[KH:K, gof : gof + H]
                            rh = rhs_t[t][KH:K, col : col + S]
                        else:
                            lh = w_sb[:, gof : gof + H]
                            rh = rhs_t[t][:, col : col + S]
                        nc.tensor.matmul(
                            dst[pb : pb + 64, dsec * S : (dsec + 1) * S], lh, rh
                        )
                nc.scalar.activation(
                    sg[:, p * 3 * S : (p + 1) * 3 * S], ifo[:, :], AF.Sigmoid
                )
                nc.scalar.activation(
                    thg[:, p * S : (p + 1) * S], pg[:, :], AF.Tanh
                )

            # strided chunk views: sg gates at chunks {i:0,3 f:1,4 o:2,5}
            sgr = sg[:, :].rearrange("p (n s) -> p n s", s=S)
            thgr = thg[:, :].rearrange("p (n s) -> p n s", s=S)
            i_v = sgr[:, 0 : 3 * npair : 3, :]
            f_v = sgr[:, 1 : 3 * npair : 3, :]

            if t == 0:
                nc.vector.tensor_mul(c_t[:, :].rearrange("p (n s) -> p n s", s=S),
                                     i_v, thgr[:, :, :])
            else:
                ig = igp.tile([128, Wg], BF16, name="ig", tag="ig")
                nc.vector.tensor_mul(
                    ig[:, :].rearrange("p (n s) -> p n s", s=S), i_v, thgr[:, :, :]
                )
                fc = fcpool.tile([128, Wg], CDT, name="fc", tag="fc")
                nc.gpsimd.tensor_mul(
                    fc[:, :].rearrange("p (n s) -> p n s", s=S), f_v,
                    c_t[:, :].rearrange("p (n s) -> p n s", s=S),
                )
                nc.vector.tensor_add(c_t[:, :], ig[:, :], fc[:, :])

            thc = thcp.tile([128, Wg], BF16, name="thc", tag="thc")
            nc.scalar.activation(thc[:, :], c_t[:, :], AF.Tanh)

            # h = sigmoid(o) * tanh(c)
            o_v = sgr[:, 2 : 3 * npair : 3, :]
            thcr = thc[:, :].rearrange("p (n s) -> p n s", s=S)
            if t == T - 1:
                dst_even = hstore[0:H, cb0 * S : cb0 * S + W].rearrange(
                    "p (n s) -> p n s", s=S
                )[:, 0 : 2 * npair : 2, :]
            else:
                dst_even = rhs_t[t + 1][0:KH, :].rearrange(
                    "p (n s) -> p n s", s=S
                )[:, 0 : 2 * npair : 2, :]
            nc.vector.tensor_mul(dst_even, o_v[0:64, :, :], thcr[0:64, :, :])
            # odd blocks: compute at partitions 64:128 then DMA down
            ht = htp.tile([128, Wg], BF16, name="ht", tag="ht")
            nc.vector.tensor_mul(
                ht[64:128, :].rearrange("p (n s) -> p n s", s=S),
                o_v[64:128, :, :], thcr[64:128, :, :],
            )
            if t == T - 1:
                dst_odd = hstore[0:H, cb0 * S : cb0 * S + W].rearrange(
                    "p (n s) -> p n s", s=S
                )[:, 1 : 2 * npair : 2, :]
            else:
                dst_odd = rhs_t[t + 1][0:KH, :].rearrange(
                    "p (n s) -> p n s", s=S
                )[:, 1 : 2 * npair : 2, :]
            nc.sync.dma_start(
                out=dst_odd,
                in_=ht[64:128, :].rearrange("p (n s) -> p n s", s=S),
            )

        def alloc_rhs(st, t):
            cb0, npair = st["cb0"], st["np"]
            W = npair * 2 * S
            st["rhs"][t] = rhsp.tile([K, W], BF16, name="rhs", tag="rhs")
            nc.gpsimd.dma_start(
                out=st["rhs"][t][KH:K, :],
                in_=xin[:, t, cb0 * S : cb0 * S + W],
            )

        for g0 in range(0, len(_COUPLES), GROUP):
            grp = _COUPLES[g0 : g0 + GROUP]
            states = []
            for cb0, npair in grp:
                c_t = cpool.tile([128, npair * S], CDT, name="c_t", tag="c_t")
                states.append({"cb0": cb0, "np": npair, "rhs": [None] * T, "c": c_t})
                for t in range(PREFETCH):
                    alloc_rhs(states[-1], t)
            for t in range(T):
                for st in states:
                    if t + PREFETCH < T:
                        alloc_rhs(st, t + PREFETCH)
                    emit_step(st, t)
            # FC for this group's blocks; overlaps the next group's ramp-up
            for st in states:
                for j in range(2 * st["np"]):
                    gc = (st["cb0"] + j) * S
                    pf = pgp.tile([128, S], F32, tag="pg", name="pf")
                    nc.tensor.matmul(
                        pf[0:C, 0:S], wfc_sb[:, :], hstore[:, gc : gc + S]
                    )
                    yt = ysp.tile([C, S], F32, name="yt", tag="yt")
                    nc.vector.tensor_copy(yt[:, :], pf[0:C, 0:S])
                    nc.sync.dma_start(out=y[:, gc : gc + S], in_=yt[:, :])

    nc.compile()
    return nc


def prep_inputs(x, W_ih, W_hh, b_ih, b_hh, W_fc, b_fc):
    """Host-side shard + transpose + weight packing. Returns in_maps."""
    x = np.asarray(x, dtype=np.float32)
    W_ih = np.asarray(W_ih, dtype=np.float32)
    W_hh = np.asarray(W_hh, dtype=np.float32)
    b = np.asarray(b_ih, dtype=np.float32) + np.asarray(b_hh, dtype=np.float32)
    W_fc = np.asarray(W_fc, dtype=np.float32)
    b_fc = np.asarray(b_fc, dtype=np.float32)

    wg = np.zeros((K, G4), dtype=np.float32)
    for g in range(4):
        rows = slice(H * g, H * g + H)
        wg[0:KH, H * g : H * g + H] = W_hh[rows, :].T
        wg[KH : KH + C, H * g : H * g + H] = W_ih[rows, :].T
        wg[K - 1, H * g : H * g + H] = b[rows]
    wg = wg.astype(NPBF16)

    wfc = np.concatenate([W_fc.T, b_fc[None, :]], axis=0).astype(NPBF16)  # (65, 8)

    bpc = x.shape[0] // NCORES
    in_maps = []
    for k in range(NCORES):
        xc = x[k * bpc : (k + 1) * bpc]              # (bpc, T, N, C)
        xt = xc.transpose(3, 1, 0, 2).reshape(C, T, SEQS)
        xext = np.zeros((KX, T, SEQS_PAD), dtype=NPBF16)
        xext[0:C, :, 0:SEQS] = xt.astype(NPBF16)
        xext[C] = np.ones((T, SEQS_PAD), dtype=NPBF16)
        in_maps.append({"xin": xext, "wg": wg, "wfc": wfc})
    return in_maps


_CACHE = {}


def _get_nc():
    if "nc" not in _CACHE:
        _CACHE["nc"] = build_nc()
    return _CACHE["nc"]


def kernel(x, W_ih, W_hh, b_ih, b_hh, W_fc, b_fc, **run_kwargs):
    nc = _get_nc()
    in_maps = prep_inputs(x, W_ih, W_hh, b_ih, b_hh, W_fc, b_fc)
    res = run_bass_kernel_spmd(nc, in_maps, list(range(NCORES)), **run_kwargs)
    outs = res.results
    ys = []
    for k in range(NCORES):
        yk = np.asarray(outs[k]["y"])               # (C, SEQS_PAD) f32
        ys.append(yk[:, 0:SEQS].T.reshape(BPC, N, C))
    y = np.concatenate(ys, axis=0)                  # (B, N, C)
    if run_kwargs.get("trace"):
        _CACHE["last_result"] = res
    return y.astype(np.float32)


# revision 6
# speedup vs baseline: 1.0088x; 1.0088x over previous
"""Trainium2 Bass kernel for nn_BasicLSTM: (B,T,N,C) shared-weight LSTM -> FC.

Strategy (data parallel over 8 cores, B=64 -> 8 batches/core):
  - seqs = 8*1370 = 10960 (padded to 11264) independent (b,n) sequences per
    core, T=12, C=8, H=64.
  - Layout "gates on partitions, sequences on free dim". Per step t, per
    sequence-block pair (two blocks of S=512 seqs sharing one rhs supertile),
    8 matmuls (4 gates x 2 blocks) with stationary lhsT = [W_hh.T; W_ih.T; b]
    (73 x 64) compute gate pre-activations into gate-major PSUM: block0 ->
    partitions 0:64, block1 -> partitions 64:128.
  - ACT: one Sigmoid spanning the [i|f|o] PSUM banks (128,1536), one Tanh on
    the g bank; tanh(c) runs on ACT for most steps but a tunable fraction is
    evicted to the DVE as a clamped (3,4)-rational (custom fused DVE ops +
    reciprocal_approx_fast; max abs err 4.2e-4).
  - DVE: i*g, c=ig+fc, o*tanh(c). GpSimd (Pool): f*c (off the DVE) and the
    per-pair-step x DMA (one (9,1024) transfer into the rhs supertile).
  - h is written by DVE directly into the next step's rhs supertile (block0);
    block1's h (partitions 64:128) moves via one SBUF->SBUF DMA.
  - x arrives pre-transposed from host as (9, T, seqs_pad) bf16 (channel 8 is
    the constant 1.0 row that carries the biases through the contraction).
  - FC tail: y = W_fc @ h_T + b_fc as K=65 matmuls over the stored final h.
"""

import os
import re
from contextlib import ExitStack

import numpy as np

import concourse.bass as bass
import concourse.mybir as mybir
import concourse.tile as tile
from concourse import bacc
from concourse.bass_utils import run_bass_kernel_spmd

B, T, N, C, H = 64, 12, 1370, 8, 64
NCORES = 8
BPC = B // NCORES          # batches per core
SEQS = BPC * N             # 10960 sequences per core
S = 512                    # sequence block size (free dim per matmul)
NBLK = 22
SEQS_PAD = NBLK * S        # 11264
KH = H                     # 64 rows of h in rhs
KX = C + 1                 # 8 x-channels + ones row
K = KH + KX                # 73
G4 = 4 * H                 # 256
NPAIR = NBLK // 2          # 11

BF16 = mybir.dt.bfloat16
F32 = mybir.dt.float32
CDT = BF16
NPBF16 = mybir.dt.np(BF16)

AF = mybir.ActivationFunctionType

# fraction of (pair, t) steps whose tanh(c) runs on DVE instead of ACT
EV_NUM, EV_DEN = 1, 4

# --- custom fused DVE ops: clamped (3,4)-rational tanh ---------------------
# tanh(x) ~= xc*(b1*t + b0) / (t^2 + a1*t + a0), t = xc^2, xc = clip(x, +-4)
# minimax fit on [0, inf): max abs err 4.2e-4
TB = 4.0
TB1, TB0 = 13.18045519, 165.69889171
TA1, TA0 = 67.812019, 165.982588


def _register_tanh_ops():
    from concourse import dve_ops as dve_ops_mod
    from concourse.dve_spec import (
        C0, C1, C2, Spec, Src0, Src1, Zero, maxx, minn, sq,
    )

    def clip(x):
        # imm2 carries -B; maxx reads it at stage 0, minn reads the derived
        # +B latch at stage 1 (SUBTRACT latches must be read deeper than 0)
        return minn(maxx(x, C2), Zero - C2)

    def make(name, spec):
        for o in dve_ops_mod.OPS:
            if o.name == name:
                return o
        row = dve_ops_mod._CUSTOM_DVE_ROW_BASE + len(dve_ops_mod.OPS)
        assert row < 0x20
        dve_ops_mod._SUB_OPCODE_FOR_NAME[name] = row
        op = dve_ops_mod.DveOp(name, spec, subdim=False, uops_sha={})
        try:
            op.compile("v3")
        except ValueError as e:
            m = re.search(r"\(v3: ([0-9a-f]+)", str(e))
            if not m:
                raise
            op = dve_ops_mod.DveOp(
                name, spec, subdim=False, uops_sha={"v3": m.group(1)}
            )
            op.compile("v3")
        dve_ops_mod.OPS.append(op)
        dve_ops_mod.CUSTOM_DVE_SPECS[name] = spec
        return op

    def _np_clip(x, b):
        return np.clip(x.astype(np.float32), -b, b)

    # den = t^2 + a1*t + a0 = (t + C0)*t + C1   (shared subtree objects —
    # the lowering CSEs by equality but hoists latches by identity)
    _xc1 = clip(Src0)
    _t1 = sq(_xc1)
    den_spec = Spec(
        body=(_t1 + C0) * _t1 + C1,
        reference=lambda in0, s0, s1, imm2: (
            (_np_clip(in0, -imm2) ** 2 + s0) * _np_clip(in0, -imm2) ** 2 + s1
        ),
    )
    # num*r = (t*C0 + C1) * xc * r
    _xc2 = clip(Src0)
    _t2 = sq(_xc2)
    fin_spec = Spec(
        body=((_t2 * C0 + C1) * _xc2) * Src1,
        reference=lambda in0, in1, s0, s1, imm2: (
            (_np_clip(in0, -imm2) ** 2 * s0 + s1) * _np_clip(in0, -imm2) * in1
        ),
    )
    return make("LSTM_TANH_DEN_ANT", den_spec), make("LSTM_TANH_FIN_ANT", fin_spec)


_TANH_DEN, _TANH_FIN = _register_tanh_ops()


def build_nc() -> bass.Bass:
    nc = bacc.Bacc("TRN2", target_bir_lowering=False, debug=False)

    xin = nc.declare_dram_parameter("xin", [KX, T, SEQS_PAD], BF16, isOutput=False)
    wg = nc.declare_dram_parameter("wg", [K, G4], BF16, isOutput=False)
    wfc = nc.declare_dram_parameter("wfc", [H + 1, C], BF16, isOutput=False)
    y = nc.declare_dram_parameter("y", [C, SEQS_PAD], F32, isOutput=True)

    with tile.TileContext(nc) as tc, ExitStack() as ctx:
        const = ctx.enter_context(tc.tile_pool(name="const", bufs=1))
        rhsp = ctx.enter_context(tc.tile_pool(name="rhs", bufs=40))
        sgp = ctx.enter_context(tc.tile_pool(name="sg", bufs=8))
        thgp = ctx.enter_context(tc.tile_pool(name="thg", bufs=8))
        thcp = ctx.enter_context(tc.tile_pool(name="thc", bufs=8))
        igp = ctx.enter_context(tc.tile_pool(name="ig", bufs=6))
        fcpool = ctx.enter_context(tc.tile_pool(name="fcp", bufs=6))
        cpool = ctx.enter_context(tc.tile_pool(name="cst", bufs=8))
        htp = ctx.enter_context(tc.tile_pool(name="htmp", bufs=8))
        denp = ctx.enter_context(tc.tile_pool(name="den", bufs=3))
        rcpp = ctx.enter_context(tc.tile_pool(name="rcp", bufs=3))
        ysp = ctx.enter_context(tc.tile_pool(name="ys", bufs=4))
        pifo = ctx.enter_context(tc.tile_pool(name="pifo", bufs=2, space="PSUM"))
        pgp = ctx.enter_context(tc.tile_pool(name="pg", bufs=2, space="PSUM"))

        w_sb = const.tile([K, G4], BF16)
        nc.sync.dma_start(out=w_sb[:, :], in_=wg[:, :])
        wfc_sb = const.tile([H + 1, C], BF16)
        nc.sync.dma_start(out=wfc_sb[:, :], in_=wfc[:, :])
        hstore = const.tile([H + 1, SEQS_PAD], BF16)
        # ones row from xin's ones channel (avoids a slow gpsimd memset)
        nc.sync.dma_start(out=hstore[H : H + 1, :], in_=xin[C : C + 1, 0, :])
        # PE warm-up + early ACT table load (sigmoid first: its table set
        # contains tanh, so no reload later), under the initial DMA shadow
        scratch = const.tile([128, S], BF16)
        nc.vector.memset(scratch[:, :], 1.0)
        nc.scalar.activation(scratch[0:1, 0:8], scratch[0:1, 0:8], AF.Sigmoid)

        WI, WF, WG, WO = 0, H, 2 * H, 3 * H

        evct = [0]

        def emit_step(st, t):
            pidx, rhs_t, c_t = st["pidx"], st["rhs"], st["c"]
            ifo = pifo.tile([128, 3 * S], F32, name="ifo", tag="ifo")
            pg = pgp.tile([128, S], F32, name="pg", tag="pg")

            # matmuls; g gate first so its tanh can start early
            for gof, dst, dsec in (
                (WG, pg, 0),
                (WI, ifo, 0),
                (WF, ifo, 1),
                (WO, ifo, 2),
            ):
                for blk in range(2):
                    pb = 64 * blk
                    col = blk * S
                    if t == 0:
                        lh = w_sb[KH:K, gof : gof + H]
                        rh = rhs_t[t][KH:K, col : col + S]
                    else:
                        lh = w_sb[:, gof : gof + H]
                        rh = rhs_t[t][:, col : col + S]
                    nc.tensor.matmul(
                        dst[pb : pb + 64, dsec * S : dsec * S + S], lh, rh
                    )

            sg = sgp.tile([128, 3 * S], BF16, name="sg", tag="sg")
            nc.scalar.activation(sg[:, :], ifo[:, :], AF.Sigmoid)
            thg = thgp.tile([128, S], BF16, name="thg", tag="thg")
            nc.scalar.activation(thg[:, :], pg[:, :], AF.Tanh)

            if t == 0:
                nc.vector.tensor_mul(c_t[:, :], sg[:, 0:S], thg[:, :])
            else:
                ig = igp.tile([128, S], BF16, name="ig", tag="ig")
                nc.vector.tensor_mul(ig[:, :], sg[:, 0:S], thg[:, :])
                fc = fcpool.tile([128, S], CDT, name="fc", tag="fc")
                nc.gpsimd.tensor_mul(fc[:, :], sg[:, S : 2 * S], c_t[:, :])
                nc.vector.tensor_add(c_t[:, :], ig[:, :], fc[:, :])

            thc = thcp.tile([128, S], BF16, name="thc", tag="thc")
            k = evct[0]
            evct[0] += 1
            if (k * EV_NUM) // EV_DEN != ((k + 1) * EV_NUM) // EV_DEN:
                den = denp.tile([128, S], F32, name="den", tag="den")
                nc.vector._custom_dve(
                    _TANH_DEN, out=den[:, :], in0=c_t[:, :],
                    s0=TA1, s1=TA0, imm2=-TB,
                )
                rcp = rcpp.tile([128, S], F32, name="rcp", tag="rcp")
                nc.vector.reciprocal_approx_fast(out=rcp[:, :], in_=den[:, :])
                nc.vector._custom_dve(
                    _TANH_FIN, out=thc[:, :], in0=c_t[:, :], in1=rcp[:, :],
                    s0=TB1, s1=TB0, imm2=-TB,
                )
            else:
                nc.scalar.activation(thc[:, :], c_t[:, :], AF.Tanh)

            # h = sigmoid(o) * tanh(c)
            if t == T - 1:
                dst0 = hstore[0:H, 2 * pidx * S : 2 * pidx * S + S]
                dst1 = hstore[0:H, (2 * pidx + 1) * S : (2 * pidx + 1) * S + S]
            else:
                dst0 = rhs_t[t + 1][0:KH, 0:S]
                dst1 = rhs_t[t + 1][0:KH, S : 2 * S]
            nc.vector.tensor_mul(dst0, sg[0:64, 2 * S : 3 * S], thc[0:64, :])
            ht = htp.tile([128, S], BF16, name="ht", tag="ht")
            nc.vector.tensor_mul(
                ht[64:128, :], sg[64:128, 2 * S : 3 * S], thc[64:128, :]
            )
            nc.sync.dma_start(out=dst1, in_=ht[64:128, :])

        GROUP = 6
        PREFETCH = 4

        def alloc_rhs(st, t):
            st["rhs"][t] = rhsp.tile([K, 2 * S], BF16, name="rhs", tag="rhs")
            col = 2 * st["pidx"] * S
            nc.gpsimd.dma_start(
                out=st["rhs"][t][KH:K, :],
                in_=xin[:, t, col : col + 2 * S],
            )

        for g0 in range(0, NPAIR, GROUP):
            grp = list(range(g0, min(g0 + GROUP, NPAIR)))
            states = []
            for pidx in grp:
                c_t = cpool.tile([128, S], CDT, name="c_t", tag="c_t")
                states.append({"pidx": pidx, "rhs": [None] * T, "c": c_t})
                for t in range(PREFETCH):
                    alloc_rhs(states[-1], t)
            for t in range(T):
                for st in states:
                    if t + PREFETCH < T:
                        alloc_rhs(st, t + PREFETCH)
                    emit_step(st, t)
            # FC for this group's blocks; overlaps the next group's ramp-up
            for st in states:
                for blk in range(2):
                    gc = (2 * st["pidx"] + blk) * S
                    pf = pgp.tile([128, S], F32, tag="pg", name="pf")
                    nc.tensor.matmul(
                        pf[0:C, 0:S], wfc_sb[:, :], hstore[:, gc : gc + S]
                    )
                    yt = ysp.tile([C, S], F32, name="yt", tag="yt")
                    nc.vector.tensor_copy(yt[:, :], pf[0:C, 0:S])
                    nc.sync.dma_start(out=y[:, gc : gc + S], in_=yt[:, :])

    nc.compile()
    return nc


def prep_inputs(x, W_ih, W_hh, b_ih, b_hh, W_fc, b_fc):
    """Host-side shard + transpose + weight packing. Returns in_maps."""
    x = np.asarray(x, dtype=np.float32)
    W_ih = np.asarray(W_ih, dtype=np.float32)
    W_hh = np.asarray(W_hh, dtype=np.float32)
    b = np.asarray(b_ih, dtype=np.float32) + np.asarray(b_hh, dtype=np.float32)
    W_fc = np.asarray(W_fc, dtype=np.float32)
    b_fc = np.asarray(b_fc, dtype=np.float32)

    wg = np.zeros((K, G4), dtype=np.float32)
    for g in range(4):
        rows = slice(H * g, H * g + H)
        wg[0:KH, H * g : H * g + H] = W_hh[rows, :].T
        wg[KH : KH + C, H * g : H * g + H] = W_ih[rows, :].T
        wg[K - 1, H * g : H * g + H] = b[rows]
    wg = wg.astype(NPBF16)

    wfc = np.concatenate([W_fc.T, b_fc[None, :]], axis=0).astype(NPBF16)  # (65, 8)

    bpc = x.shape[0] // NCORES
    in_maps = []
    for k in range(NCORES):
        xc = x[k * bpc : (k + 1) * bpc]              # (bpc, T, N, C)
        xt = xc.transpose(3, 1, 0, 2).reshape(C, T, SEQS)
        xext = np.zeros((KX, T, SEQS_PAD), dtype=NPBF16)
        xext[0:C, :, 0:SEQS] = xt.astype(NPBF16)
        xext[C] = np.ones((T, SEQS_PAD), dtype=NPBF16)
        in_maps.append({"xin": xext, "wg": wg, "wfc": wfc})
    return in_maps


_CACHE = {}


def _get_nc():
    if "nc" not in _CACHE:
        _CACHE["nc"] = build_nc()
    return _CACHE["nc"]


def kernel(x, W_ih, W_hh, b_ih, b_hh, W_fc, b_fc, **run_kwargs):
    nc = _get_nc()
    in_maps = prep_inputs(x, W_ih, W_hh, b_ih, b_hh, W_fc, b_fc)
    res = run_bass_kernel_spmd(nc, in_maps, list(range(NCORES)), **run_kwargs)
    outs = res.results
    ys = []
    for k in range(NCORES):
        yk = np.asarray(outs[k]["y"])               # (C, SEQS_PAD) f32
        ys.append(yk[:, 0:SEQS].T.reshape(BPC, N, C))
    y = np.concatenate(ys, axis=0)                  # (B, N, C)
    if run_kwargs.get("trace"):
        _CACHE["last_result"] = res
    return y.astype(np.float32)


# revision 7
# speedup vs baseline: 1.1278x; 1.1179x over previous
"""Trainium2 Bass kernel for nn_BasicLSTM: (B,T,N,C) shared-weight LSTM -> FC.

Strategy (data parallel over 8 cores, B=64 -> 8 batches/core):
  - seqs = 8*1370 = 10960 (padded to 11264) independent (b,n) sequences per
    core, T=12, C=8, H=64.
  - Layout "gates on partitions, sequences on free dim". Per step t, per
    sequence-block pair (two blocks of S=512 seqs sharing one rhs supertile),
    8 matmuls (4 gates x 2 blocks) with stationary lhsT = [W_hh.T; W_ih.T; b]
    (73 x 64) compute gate pre-activations into gate-major PSUM: block0 ->
    partitions 0:64, block1 -> partitions 64:128.
  - ACT: one Sigmoid spanning the [i|f|o] PSUM banks (128,1536), one Tanh on
    the g bank; tanh(c) runs on ACT for most steps but a tunable fraction is
    evicted to the DVE as a clamped (3,4)-rational (custom fused DVE ops +
    reciprocal_approx_fast; max abs err 4.2e-4).
  - DVE: i*g, c=ig+fc, o*tanh(c). GpSimd (Pool): f*c (off the DVE) and the
    per-pair-step x DMA (one (9,1024) transfer into the rhs supertile).
  - h is written by DVE directly into the next step's rhs supertile (block0);
    block1's h (partitions 64:128) moves via one SBUF->SBUF DMA.
  - x arrives pre-transposed from host as (9, T, seqs_pad) bf16 (channel 8 is
    the constant 1.0 row that carries the biases through the contraction).
  - FC tail: y = W_fc @ h_T + b_fc as K=65 matmuls over the stored final h.
"""

import os
import re
from contextlib import ExitStack

import numpy as np

import concourse.bass as bass
import concourse.mybir as mybir
import concourse.tile as tile
from concourse import bacc
from concourse.bass_utils import run_bass_kernel_spmd

B, T, N, C, H = 64, 12, 1370, 8, 64
NCORES = 8
BPC = B // NCORES          # batches per core
SEQS = BPC * N             # 10960 sequences per core
S = 512                    # sequence block size (free dim per matmul)
NBLK = 22
SEQS_PAD = NBLK * S        # 11264
KH = H                     # 64 rows of h in rhs
KX = C + 1                 # 8 x-channels + ones row
K = KH + KX                # 73
G4 = 4 * H                 # 256
NPAIR = NBLK // 2          # 11

BF16 = mybir.dt.bfloat16
F32 = mybir.dt.float32
CDT = BF16
NPBF16 = mybir.dt.np(BF16)

AF = mybir.ActivationFunctionType

# fraction of (pair, t) steps whose tanh(c) runs on DVE instead of ACT
EV_NUM, EV_DEN = 1, 4

# --- custom fused DVE ops: clamped (3,4)-rational tanh ---------------------
# tanh(x) ~= xc*(b1*t + b0) / (t^2 + a1*t + a0), t = xc^2, xc = clip(x, +-4)
# minimax fit on [0, inf): max abs err 4.2e-4
TB = 4.0
TB1, TB0 = 13.18045519, 165.69889171
TA1, TA0 = 67.812019, 165.982588


def _register_tanh_ops():
    from concourse import dve_ops as dve_ops_mod
    from concourse.dve_spec import (
        C0, C1, C2, Spec, Src0, Src1, Zero, maxx, minn, sq,
    )

    def clip(x):
        # imm2 carries -B; maxx reads it at stage 0, minn reads the derived
        # +B latch at stage 1 (SUBTRACT latches must be read deeper than 0)
        return minn(maxx(x, C2), Zero - C2)

    def make(name, spec):
        for o in dve_ops_mod.OPS:
            if o.name == name:
                return o
        row = dve_ops_mod._CUSTOM_DVE_ROW_BASE + len(dve_ops_mod.OPS)
        assert row < 0x20
        dve_ops_mod._SUB_OPCODE_FOR_NAME[name] = row
        op = dve_ops_mod.DveOp(name, spec, subdim=False, uops_sha={})
        try:
            op.compile("v3")
        except ValueError as e:
            m = re.search(r"\(v3: ([0-9a-f]+)", str(e))
            if not m:
                raise
            op = dve_ops_mod.DveOp(
                name, spec, subdim=False, uops_sha={"v3": m.group(1)}
            )
            op.compile("v3")
        dve_ops_mod.OPS.append(op)
        dve_ops_mod.CUSTOM_DVE_SPECS[name] = spec
        return op

    def _np_clip(x, b):
        return np.clip(x.astype(np.float32), -b, b)

    # den = t^2 + a1*t + a0 = (t + C0)*t + C1   (shared subtree objects —
    # the lowering CSEs by equality but hoists latches by identity)
    _xc1 = clip(Src0)
    _t1 = sq(_xc1)
    den_spec = Spec(
        body=(_t1 + C0) * _t1 + C1,
        reference=lambda in0, s0, s1, imm2: (
            (_np_clip(in0, -imm2) ** 2 + s0) * _np_clip(in0, -imm2) ** 2 + s1
        ),
    )
    # num*r = (t*C0 + C1) * xc * r
    _xc2 = clip(Src0)
    _t2 = sq(_xc2)
    fin_spec = Spec(
        body=((_t2 * C0 + C1) * _xc2) * Src1,
        reference=lambda in0, in1, s0, s1, imm2: (
            (_np_clip(in0, -imm2) ** 2 * s0 + s1) * _np_clip(in0, -imm2) * in1
        ),
    )
    return make("LSTM_TANH_DEN_ANT", den_spec), make("LSTM_TANH_FIN_ANT", fin_spec)


_TANH_DEN, _TANH_FIN = _register_tanh_ops()


def build_nc() -> bass.Bass:
    nc = bacc.Bacc("TRN2", target_bir_lowering=False, debug=False)

    xin = nc.declare_dram_parameter("xin", [KX, T, SEQS_PAD], BF16, isOutput=False)
    wg = nc.declare_dram_parameter("wg", [K, G4], BF16, isOutput=False)
    wfc = nc.declare_dram_parameter("wfc", [H + 1, C], BF16, isOutput=False)
    y = nc.declare_dram_parameter("y", [C, SEQS_PAD], F32, isOutput=True)

    with tile.TileContext(nc) as tc, ExitStack() as ctx:
        const = ctx.enter_context(tc.tile_pool(name="const", bufs=1))
        rhsp = ctx.enter_context(tc.tile_pool(name="rhs", bufs=40))
        sgp = ctx.enter_context(tc.tile_pool(name="sg", bufs=8))
        thgp = ctx.enter_context(tc.tile_pool(name="thg", bufs=8))
        thcp = ctx.enter_context(tc.tile_pool(name="thc", bufs=8))
        igp = ctx.enter_context(tc.tile_pool(name="ig", bufs=6))
        fcpool = ctx.enter_context(tc.tile_pool(name="fcp", bufs=6))
        cpool = ctx.enter_context(tc.tile_pool(name="cst", bufs=8))
        htp = ctx.enter_context(tc.tile_pool(name="htmp", bufs=8))
        denp = ctx.enter_context(tc.tile_pool(name="den", bufs=3))
        rcpp = ctx.enter_context(tc.tile_pool(name="rcp", bufs=3))
        ysp = ctx.enter_context(tc.tile_pool(name="ys", bufs=4))
        pifo = ctx.enter_context(tc.tile_pool(name="pifo", bufs=2, space="PSUM"))
        pgp = ctx.enter_context(tc.tile_pool(name="pg", bufs=2, space="PSUM"))

        w_sb = const.tile([K, G4], BF16)
        nc.sync.dma_start(out=w_sb[:, :], in_=wg[:, :])
        wfc_sb = const.tile([H + 1, C], BF16)
        nc.sync.dma_start(out=wfc_sb[:, :], in_=wfc[:, :])
        hstore = const.tile([H + 1, SEQS_PAD], BF16)
        # ones row from xin's ones channel (avoids a slow gpsimd memset)
        nc.sync.dma_start(out=hstore[H : H + 1, :], in_=xin[C : C + 1, 0, :])
        # PE warm-up + early ACT table load (sigmoid first: its table set
        # contains tanh, so no reload later), under the initial DMA shadow
        scratch = const.tile([128, S], BF16)
        nc.vector.memset(scratch[:, :], 1.0)
        nc.scalar.activation(scratch[0:1, 0:8], scratch[0:1, 0:8], AF.Sigmoid)

        WI, WF, WG, WO = 0, H, 2 * H, 3 * H

        evct = [0]

        def emit_step(st, t):
            pidx, rhs_t, c_t = st["pidx"], st["rhs"], st["c"]
            ifo = pifo.tile([128, 3 * S], F32, name="ifo", tag="ifo")
            pg = pgp.tile([128, S], F32, name="pg", tag="pg")

            # matmuls; g gate first so its tanh can start early
            for gof, dst, dsec in (
                (WG, pg, 0),
                (WI, ifo, 0),
                (WF, ifo, 1),
                (WO, ifo, 2),
            ):
                for blk in range(2):
                    pb = 64 * blk
                    col = blk * S
                    if t == 0:
                        lh = w_sb[KH:K, gof : gof + H]
                        rh = rhs_t[t][KH:K, col : col + S]
                    else:
                        lh = w_sb[:, gof : gof + H]
                        rh = rhs_t[t][:, col : col + S]
                    nc.tensor.matmul(
                        dst[pb : pb + 64, dsec * S : dsec * S + S], lh, rh
                    )

            sg = sgp.tile([128, 3 * S], BF16, name="sg", tag="sg")
            nc.scalar.activation(sg[:, :], ifo[:, :], AF.Sigmoid)
            thg = thgp.tile([128, S], BF16, name="thg", tag="thg")
            nc.scalar.activation(thg[:, :], pg[:, :], AF.Tanh)

            if t == 0:
                nc.vector.tensor_mul(c_t[:, :], sg[:, 0:S], thg[:, :])
            else:
                ig = igp.tile([128, S], BF16, name="ig", tag="ig")
                nc.vector.tensor_mul(ig[:, :], sg[:, 0:S], thg[:, :])
                fc = fcpool.tile([128, S], CDT, name="fc", tag="fc")
                nc.vector.tensor_mul(fc[:, :], sg[:, S : 2 * S], c_t[:, :])
                nc.vector.tensor_add(c_t[:, :], ig[:, :], fc[:, :])

            thc = thcp.tile([128, S], BF16, name="thc", tag="thc")
            k = evct[0]
            evct[0] += 1
            if (k * EV_NUM) // EV_DEN != ((k + 1) * EV_NUM) // EV_DEN:
                den = denp.tile([128, S], F32, name="den", tag="den")
                nc.vector._custom_dve(
                    _TANH_DEN, out=den[:, :], in0=c_t[:, :],
                    s0=TA1, s1=TA0, imm2=-TB,
                )
                rcp = rcpp.tile([128, S], F32, name="rcp", tag="rcp")
                nc.vector.reciprocal_approx_fast(out=rcp[:, :], in_=den[:, :])
                nc.vector._custom_dve(
                    _TANH_FIN, out=thc[:, :], in0=c_t[:, :], in1=rcp[:, :],
                    s0=TB1, s1=TB0, imm2=-TB,
                )
            else:
                nc.scalar.activation(thc[:, :], c_t[:, :], AF.Tanh)

            # h = sigmoid(o) * tanh(c)
            if t == T - 1:
                dst0 = hstore[0:H, 2 * pidx * S : 2 * pidx * S + S]
                dst1 = hstore[0:H, (2 * pidx + 1) * S : (2 * pidx + 1) * S + S]
            else:
                dst0 = rhs_t[t + 1][0:KH, 0:S]
                dst1 = rhs_t[t + 1][0:KH, S : 2 * S]
            nc.vector.tensor_mul(dst0, sg[0:64, 2 * S : 3 * S], thc[0:64, :])
            ht = htp.tile([128, S], BF16, name="ht", tag="ht")
            nc.vector.tensor_mul(
                ht[64:128, :], sg[64:128, 2 * S : 3 * S], thc[64:128, :]
            )
            nc.sync.dma_start(out=dst1, in_=ht[64:128, :])

        GROUP = 6
        PREFETCH = 4

        def alloc_rhs(st, t):
            st["rhs"][t] = rhsp.tile([K, 2 * S], BF16, name="rhs", tag="rhs")
            col = 2 * st["pidx"] * S
            nc.gpsimd.dma_start(
                out=st["rhs"][t][KH:K, :],
                in_=xin[:, t, col : col + 2 * S],
            )

        for g0 in range(0, NPAIR, GROUP):
            grp = list(range(g0, min(g0 + GROUP, NPAIR)))
            states = []
            for pidx in grp:
                c_t = cpool.tile([128, S], CDT, name="c_t", tag="c_t")
                states.append({"pidx": pidx, "rhs": [None] * T, "c": c_t})
                for t in range(PREFETCH):
                    alloc_rhs(states[-1], t)
            for t in range(T):
                for st in states:
                    if t + PREFETCH < T:
                        alloc_rhs(st, t + PREFETCH)
                    emit_step(st, t)
            # FC for this group's blocks; overlaps the next group's ramp-up
            for st in states:
                for blk in range(2):
                    gc = (2 * st["pidx"] + blk) * S
                    pf = pgp.tile([128, S], F32, tag="pg", name="pf")
                    nc.tensor.matmul(
                        pf[0:C, 0:S], wfc_sb[:, :], hstore[:, gc : gc + S]
                    )
                    yt = ysp.tile([C, S], F32, name="yt", tag="yt")
                    nc.vector.tensor_copy(yt[:, :], pf[0:C, 0:S])
                    nc.sync.dma_start(out=y[:, gc : gc + S], in_=yt[:, :])

    nc.compile()
    return nc


def prep_inputs(x, W_ih, W_hh, b_ih, b_hh, W_fc, b_fc):
    """Host-side shard + transpose + weight packing. Returns in_maps."""
    x = np.asarray(x, dtype=np.float32)
    W_ih = np.asarray(W_ih, dtype=np.float32)
    W_hh = np.asarray(W_hh, dtype=np.float32)
    b = np.asarray(b_ih, dtype=np.float32) + np.asarray(b_hh, dtype=np.float32)
    W_fc = np.asarray(W_fc, dtype=np.float32)
    b_fc = np.asarray(b_fc, dtype=np.float32)

    wg = np.zeros((K, G4), dtype=np.float32)
    for g in range(4):
        rows = slice(H * g, H * g + H)
        wg[0:KH, H * g : H * g + H] = W_hh[rows, :].T
        wg[KH : KH + C, H * g : H * g + H] = W_ih[rows, :].T
        wg[K - 1, H * g : H * g + H] = b[rows]
    wg = wg.astype(NPBF16)

    wfc = np.concatenate([W_fc.T, b_fc[None, :]], axis=0).astype(NPBF16)  # (65, 8)

    bpc = x.shape[0] // NCORES
    in_maps = []
    for k in range(NCORES):
        xc = x[k * bpc : (k + 1) * bpc]              # (bpc, T, N, C)
        xt = xc.transpose(3, 1, 0, 2).reshape(C, T, SEQS)
        xext = np.zeros((KX, T, SEQS_PAD), dtype=NPBF16)
        xext[0:C, :, 0:SEQS] = xt.astype(NPBF16)
        xext[C] = np.ones((T, SEQS_PAD), dtype=NPBF16)
        in_maps.append({"xin": xext, "wg": wg, "wfc": wfc})
    return in_maps


_CACHE = {}


def _get_nc():
    if "nc" not in _CACHE:
        _CACHE["nc"] = build_nc()
    return _CACHE["nc"]


def kernel(x, W_ih, W_hh, b_ih, b_hh, W_fc, b_fc, **run_kwargs):
    nc = _get_nc()
    in_maps = prep_inputs(x, W_ih, W_hh, b_ih, b_hh, W_fc, b_fc)
    res = run_bass_kernel_spmd(nc, in_maps, list(range(NCORES)), **run_kwargs)
    outs = res.results
    ys = []
    for k in range(NCORES):
        yk = np.asarray(outs[k]["y"])               # (C, SEQS_PAD) f32
        ys.append(yk[:, 0:SEQS].T.reshape(BPC, N, C))
    y = np.concatenate(ys, axis=0)                  # (B, N, C)
    if run_kwargs.get("trace"):
        _CACHE["last_result"] = res
    return y.astype(np.float32)
